# revision 45
# baseline (speedup 1.0000x reference)
"""Fused pre-LN multi-head attention (B=4, S=2048, D=1024, H=16) on 8 trn2 cores.

Sharding: core c -> batch b = c // 2, query-half = c % 2. Each core receives
the FULL 2048-row sequence of its batch (host pre-swapped so the core's own
1024 query rows are always seq tiles 0..7, cast to bf16). The core runs
LayerNorm and K/V projections over all 2048 rows (K/V computed redundantly on
both cores of a batch pair -- cheaper than exchanging K/V via a collective),
the Q projection over its local 1024 rows, attention (16 heads) over the
local queries with full-sequence K/V, and the output projection for its 1024
rows. The host concatenates. No collectives anywhere.

Attention is software-pipelined at seq-tile granularity: while the ctx
matmuls of head-pair t accumulate (two staggered psum groups per head so
probs tiles free at the exp consumption rate), the scores + exp of head-pair
t+1 are emitted, keeping both the PE and Activation engines busy.

LayerNorm gamma/beta and the 1/sqrt(head_dim) scale are folded into the
(host-pre-transposed, bf16) projection weights. Softmax skips max-subtraction
(scores are O(1) by construction); the denominator comes from a ones-column
appended to V inside the probs @ V matmul.
"""

import numpy as np
import ml_dtypes

import concourse.bass as bass
import concourse.mybir as mybir
import concourse.tile as tile
from concourse import bacc
from concourse.bass import ds
from concourse.bass_utils import run_bass_kernel_spmd

F32 = mybir.dt.float32
BF16 = mybir.dt.bfloat16

B, S, D = 4, 2048, 1024
H, HD = 16, 64
EPS = 1e-6
P = 128
NDT = D // P          # 8  d-tiles
NST = S // P          # 16 seq tiles (full batch sequence)
QROWS = S // 2        # 1024 query rows per core
NQT = QROWS // P      # 8
NLT = NST // 2        # 8  local (query) seq tiles
NCORES = 8
HP = H // 2           # 8 head pairs
VSTRIDE = HD + 1      # 65: per-head V columns incl. the ones column


def build_program(nrep=1):
    nc = bacc.Bacc("TRN2", target_bir_lowering=False)

    x_d = nc.dram_tensor("x", [S, D], BF16, kind="ExternalInput")
    wqt_d = nc.dram_tensor("wqt", [D, D], BF16, kind="ExternalInput")
    wkt_d = nc.dram_tensor("wkt", [D, D], BF16, kind="ExternalInput")
    wvt_d = nc.dram_tensor("wvt", [D, D], BF16, kind="ExternalInput")
    wot_d = nc.dram_tensor("wot", [D, D], BF16, kind="ExternalInput")
    id_d = nc.dram_tensor("ident", [P, P], BF16, kind="ExternalInput")
    bq_d = nc.dram_tensor("bq", [P, NDT], F32, kind="ExternalInput")
    bk_d = nc.dram_tensor("bk", [P, NDT], F32, kind="ExternalInput")
    bv_d = nc.dram_tensor("bv", [1, D], F32, kind="ExternalInput")
    bo_d = nc.dram_tensor("bo", [1, D], F32, kind="ExternalInput")
    out_d = nc.dram_tensor("out", [QROWS, D], F32, kind="ExternalOutput")

    sub, mult, add = (
        mybir.AluOpType.subtract,
        mybir.AluOpType.mult,
        mybir.AluOpType.add,
    )
    AF = mybir.ActivationFunctionType

    with tile.TileContext(nc) as tc:
        with (
            tc.tile_pool(name="consts", bufs=1) as consts,
            tc.tile_pool(name="qt", bufs=1) as qt_pool,
            tc.tile_pool(name="kt", bufs=1) as kt_pool,
            tc.tile_pool(name="vp", bufs=1) as v_pool,
            tc.tile_pool(name="ctxt", bufs=1) as ct_pool,
        ):
            eps_t = consts.tile([P, 1], F32)
            nc.vector.memset(eps_t, EPS)
            bq_t = consts.tile([P, NDT], F32)
            bk_t = consts.tile([P, NDT], F32)
            nc.gpsimd.dma_start(out=bq_t, in_=bq_d.ap())
            nc.gpsimd.dma_start(out=bk_t, in_=bk_d.ap())
            bvb = consts.tile([P, D], F32)
            nc.sync.dma_start(out=bvb, in_=bv_d.ap().to_broadcast([P, D]))
            ident = consts.tile([P, P], BF16)
            nc.sync.dma_start(out=ident, in_=id_d.ap())

            # [p, seq_tile, head, 65]; the ones column makes the ctx matmul
            # also produce the softmax denominator. Even heads: v in cols
            # 0:64, ones at 64 (denominator at psum row 64, ctx rows 0..63).
            # Odd heads: ones at 0, v in cols 1:65 -- their ctx psum group is
            # written at partition offset 63, so ctx lands lane-aligned at
            # partitions 64..127 (= its CT rows) with the denominator at 63.
            V = v_pool.tile([P, NST, H * VSTRIDE], BF16)
            Vr = V.rearrange("p s (h e) -> p s h e", e=VSTRIDE)
            nc.vector.memset(Vr[:, :, 0::2, HD : HD + 1], 1.0)
            nc.vector.memset(Vr[:, :, 1::2, 0:1], 1.0)

            QT = qt_pool.tile([P, NDT, QROWS], BF16)
            KT = kt_pool.tile([P, NDT, S], BF16)
            CT = ct_pool.tile([P, NDT, QROWS], BF16)

            # ---- LayerNorm + transpose + V/K/Q projections (full seq) ----
            for _rep in range(nrep):
              with (
                  tc.tile_pool(name="wq", bufs=1) as wq_pool,
                  tc.tile_pool(name="wk", bufs=1) as wk_pool,
                  tc.tile_pool(name="wv", bufs=1) as wv_pool,
                  tc.tile_pool(name="xp", bufs=6) as x_pool,
                  tc.tile_pool(name="xnp", bufs=4) as xn_pool,
                  tc.tile_pool(name="xntp", bufs=1) as xnt_pool,
                  tc.tile_pool(name="statp", bufs=8) as stat_pool,
                  tc.tile_pool(name="psum_proj", bufs=2, space="PSUM") as psum_proj,
                  tc.tile_pool(name="psum_tr", bufs=4, space="PSUM") as psum_tr,
              ):
                  WQ = wq_pool.tile([P, NDT, D], BF16)
                  WK = wk_pool.tile([P, NDT, D], BF16)
                  WV = wv_pool.tile([P, NDT, D], BF16)

                  XNT = xnt_pool.tile([P, NDT, S], BF16)

                  # pre-issue the first x-tile loads so LayerNorm starts
                  # before the weight transfers occupy the DMA engines
                  NPRE = 3
                  xts = []
                  for s in range(NPRE):
                      xt = x_pool.tile([P, D], BF16, name="xt")
                      nc.sync.dma_start(out=xt, in_=x_d.ap()[s * P : (s + 1) * P, :])
                      xts.append(xt)

                  for s in range(NST):
                      if s < NPRE:
                          xt = xts[s]
                      else:
                          xt = x_pool.tile([P, D], BF16, name="xt")
                          nc.sync.dma_start(out=xt, in_=x_d.ap()[s * P : (s + 1) * P, :])
                      st = stat_pool.tile([P, 2, 6], F32)
                      nc.vector.bn_stats(out=st[:, 0], in_=xt[:, 0:512])
                      nc.vector.bn_stats(out=st[:, 1], in_=xt[:, 512:1024])
                      mv = stat_pool.tile([P, 2], F32)
                      nc.vector.bn_aggr(out=mv, in_=st)
                      std = stat_pool.tile([P, 1], F32)
                      nc.scalar.activation(
                          out=std, in_=mv[:, 1:2], func=AF.Sqrt, bias=eps_t
                      )
                      rstd = stat_pool.tile([P, 1], F32)
                      nc.vector.reciprocal(out=rstd, in_=std)
                      xn = xn_pool.tile([P, D], BF16)
                      nc.vector.tensor_scalar(
                          out=xn,
                          in0=xt,
                          scalar1=mv[:, 0:1],
                          scalar2=rstd,
                          op0=sub,
                          op1=mult,
                      )
                      # transpose xn into XNT on the tensor engine (128x128
                      # blocks via identity matmul, 4 blocks batched per psum
                      # tile) with one Act-engine psum->SBUF copy per batch;
                      # keeps the DMA channel free for x and W loads
                      for jh in range(2):
                          trp = psum_tr.tile([P, 4 * P], BF16)
                          for j2 in range(4):
                              j = 4 * jh + j2
                              nc.tensor.transpose(
                                  trp[:, j2 * P : (j2 + 1) * P],
                                  xn[:, j * P : (j + 1) * P],
                                  ident,
                              )
                          nc.scalar.activation(
                              out=XNT[:, 4 * jh : 4 * jh + 4, s * P : (s + 1) * P],
                              in_=trp.rearrange("p (j c) -> p j c", c=P),
                              func=AF.Copy,
                          )
                      if s == 0:
                          for _t in range(NDT):
                              nc.gpsimd.dma_start(
                                  out=WV[:, _t, :],
                                  in_=wvt_d.ap().rearrange("(t p) j -> p t j", p=P)[:, _t, :],
                              )
                      elif s == 1:
                          for _t in range(NDT):
                              nc.gpsimd.dma_start(
                                  out=WK[:, _t, :],
                                  in_=wkt_d.ap().rearrange("(t p) j -> p t j", p=P)[:, _t, :],
                              )
                      elif s == 4:
                          for _t in range(NDT):
                              nc.gpsimd.dma_start(
                                  out=WQ[:, _t, :],
                                  in_=wqt_d.ap().rearrange("(t p) j -> p t j", p=P)[:, _t, :],
                              )
                      # V projection for this seq tile: V[s] = xn[s] @ Wv.T
                      for df in range(2):
                          ps = psum_proj.tile([P, 512], F32)
                          for k in range(NDT):
                              nc.tensor.matmul(
                                  ps,
                                  lhsT=XNT[:, k, s * P : (s + 1) * P],
                                  rhs=WV[:, k, df * 512 : (df + 1) * 512],
                                  start=(k == 0),
                                  stop=(k == NDT - 1),
                              )
                          ps_h = ps.rearrange("p (h e) -> p h e", e=HD)
                          bv_h = bvb[:, df * 512 : (df + 1) * 512].rearrange(
                              "p (h e) -> p h e", e=HD
                          )
                          nc.vector.tensor_tensor(
                              out=Vr[:, s, df * 8 : (df + 1) * 8 : 2, 0:HD],
                              in0=ps_h[:, 0::2],
                              in1=bv_h[:, 0::2],
                              op=add,
                          )
                          nc.vector.tensor_tensor(
                              out=Vr[:, s, df * 8 + 1 : (df + 1) * 8 : 2, 1 : HD + 1],
                              in0=ps_h[:, 1::2],
                              in1=bv_h[:, 1::2],
                              op=add,
                          )
                      if s % 4 == 3:
                          kf = s // 4
                          # K^T chunk (kpos columns kf*512 ..)
                          for i in range(NDT):
                              ps = psum_proj.tile([P, 512], F32)
                              for k in range(NDT):
                                  nc.tensor.matmul(
                                      ps,
                                      lhsT=WK[:, k, i * P : (i + 1) * P],
                                      rhs=XNT[:, k, kf * 512 : (kf + 1) * 512],
                                      start=(k == 0),
                                      stop=(k == NDT - 1),
                                  )
                              nc.vector.tensor_scalar(
                                  out=KT[:, i, kf * 512 : (kf + 1) * 512],
                                  in0=ps,
                                  scalar1=bk_t[:, i : i + 1],
                                  scalar2=None,
                                  op0=add,
                              )
                      if s == NLT - 1:
                          # Q projection (local query rows = tiles 0..7)
                          for i in range(NDT):
                              for qf in range(QROWS // 512):
                                  ps = psum_proj.tile([P, 512], F32)
                                  for k in range(NDT):
                                      nc.tensor.matmul(
                                          ps,
                                          lhsT=WQ[:, k, i * P : (i + 1) * P],
                                          rhs=XNT[:, k, qf * 512 : (qf + 1) * 512],
                                          start=(k == 0),
                                          stop=(k == NDT - 1),
                                      )
                                  nc.vector.tensor_scalar(
                                      out=QT[:, i, qf * 512 : (qf + 1) * 512],
                                      in0=ps,
                                      scalar1=bq_t[:, i : i + 1],
                                      scalar2=None,
                                      op0=add,
                                  )

              # ---- attention (WO prefetched so out-proj starts instantly) --
              with (
                  tc.tile_pool(name="wo", bufs=1) as wo_pool,
                  tc.tile_pool(name="bobp", bufs=1) as bob_pool,
              ):
                WO = wo_pool.tile([P, NDT, D], BF16)
                for _t in range(NDT):
                    nc.sync.dma_start(
                        out=WO[:, _t, :],
                        in_=wot_d.ap().rearrange("(t p) j -> p t j", p=P)[:, _t, :],
                    )
                bob = bob_pool.tile([P, D], F32)
                nc.sync.dma_start(out=bob, in_=bo_d.ap().to_broadcast([P, D]))
                with (
                  tc.tile_pool(name="probs", bufs=32) as probs_pool,
                  tc.tile_pool(name="sep", bufs=2) as se_pool,
                  tc.tile_pool(name="psum_sc", bufs=2, space="PSUM") as psum_sc,
                  tc.tile_pool(name="psum_cx", bufs=4, space="PSUM") as psum_cx,
                ):
                  probs_all = [
                      [[None] * NST for _ in range(2)] for _ in range(HP)
                  ]

                  def emit_scores(t, kt):
                      for hi in range(2):
                          off = hi * HD
                          sps = psum_sc.tile([P, QROWS], F32)
                          for qf in range(QROWS // 512):
                              nc.tensor.matmul(
                                  sps[:, qf * 512 : (qf + 1) * 512],
                                  lhsT=KT[off : off + HD, t, kt * P : (kt + 1) * P],
                                  rhs=QT[off : off + HD, t, qf * 512 : (qf + 1) * 512],
                                  start=True,
                                  stop=True,
                                  tile_position=(off, 0),
                              )
                          pt = probs_pool.tile([P, QROWS], BF16)
                          nc.scalar.activation(out=pt, in_=sps, func=AF.Exp)
                          probs_all[t][hi][kt] = pt

                  def finalize(t, hi, qf, cps):
                      # rows 0..63 = unnormalized ctx, row 64 = sum(exp).
                      # Drain psum immediately (recip + raw-ctx copy) so the
                      # cps slot frees for the next pair without waiting for
                      # the full normalization chain.
                      se = se_pool.tile([P, 512], F32, tag="se")
                      nc.vector.reciprocal(
                          out=se[HD : HD + 1, :],
                          in_=cps[HD : HD + 1, :],
                      )
                      raw = se_pool.tile([HD, 512], BF16, tag="raw", bufs=4)
                      nc.vector.tensor_copy(raw, cps[0:HD, :])
                      # HW partition_broadcast only reads partition 0:
                      # shift the denominator row down first via DMA.
                      se0 = se_pool.tile([1, 512], F32, tag="se0")
                      nc.gpsimd.dma_start(out=se0, in_=se[HD : HD + 1, :])
                      seb = se_pool.tile([P, 512], F32, tag="seb")
                      nc.gpsimd.partition_broadcast(seb[0:HD, :], se0)
                      if hi == 0:
                          nc.vector.tensor_tensor(
                              out=CT[0:HD, t, qf * 512 : (qf + 1) * 512],
                              in0=raw,
                              in1=seb[0:HD, :],
                              op=mult,
                          )
                      else:
                          tmp = se_pool.tile([HD, 512], BF16, tag="ctmp")
                          nc.vector.tensor_tensor(
                              out=tmp,
                              in0=raw,
                              in1=seb[0:HD, :],
                              op=mult,
                          )
                          # partition shift 0..63 -> 64..127 via DMA
                          nc.gpsimd.dma_start(
                              out=CT[HD:P, t, qf * 512 : (qf + 1) * 512],
                              in_=tmp,
                          )

                  with tc.tile_pool(name="osb", bufs=2) as osb_pool:

                    def emit_outproj(qt):
                        # shares the psum_sc ring (same tag/shape) -- by the
                        # time out-proj runs, scores emission has stopped, so
                        # the ring slots alternate between out-proj q-tiles
                        ops = psum_sc.tile([P, QROWS], F32, name="sps")
                        for jf in range(2):
                            ps = ops[:, jf * 512 : (jf + 1) * 512]
                            for i in range(NDT):
                                nc.tensor.matmul(
                                    ps,
                                    lhsT=CT[:, i, qt * P : (qt + 1) * P],
                                    rhs=WO[:, i, jf * 512 : (jf + 1) * 512],
                                    start=(i == 0),
                                    stop=(i == NDT - 1),
                                )
                            ot = osb_pool.tile([P, 512], F32)
                            nc.vector.tensor_tensor(
                                out=ot,
                                in0=ps,
                                in1=bob[:, jf * 512 : (jf + 1) * 512],
                                op=add,
                            )
                            nc.sync.dma_start(
                                out=out_d.ap()[
                                    qt * P : (qt + 1) * P,
                                    jf * 512 : (jf + 1) * 512,
                                ],
                                in_=ot,
                            )

                    for kt in range(NST):
                        emit_scores(0, kt)
                    for t in range(HP):
                      probs = probs_all[t]
                      cps = [
                          [
                              psum_cx.tile(
                                  [VSTRIDE, 512], F32,
                                  name=f"cps{_hi}{_qf}", tag="cps",
                              )
                              for _qf in range(2)
                          ]
                          for _hi in range(2)
                      ]
                      # qf=1 trails qf=0 by one step so each probs tile's last
                      # read happens one step after its first, matching the
                      # exp production rate of the next pair's scores.
                      for step in range(NST + 1):
                          if step < NST and t + 1 < HP:
                              emit_scores(t + 1, step)
                          for hi in range(2):
                              for qf, kt in ((0, step), (1, step - 1)):
                                  if 0 <= kt < NST:
                                      nc.tensor.matmul(
                                          cps[hi][qf],
                                          lhsT=Vr[:, kt, 2 * t + hi, :],
                                          rhs=probs[hi][kt][
                                              :, qf * 512 : (qf + 1) * 512
                                          ],
                                          start=(kt == 0),
                                          stop=(kt == NST - 1),
                                      )
                                      if kt == NST - 1:
                                          finalize(t, hi, qf, cps[hi][qf])
                          if t == HP - 1:
                              # overlap out-proj with the last pair's tail:
                              # qf0 columns of CT finalize at step NST-1
                              if step == NST - 1:
                                  for qt in range(NQT // 2):
                                      emit_outproj(qt)
                              elif step == NST:
                                  for qt in range(NQT // 2, NQT):
                                      emit_outproj(qt)

    nc.compile()
    return nc


_NC_CACHE = None


def _get_program():
    global _NC_CACHE
    if _NC_CACHE is None:
        _NC_CACHE = build_program()
    return _NC_CACHE


def _prep_host(x, ln_gamma, ln_beta, Wq, bq, Wk, bk, Wv, bv, Wo, bo):
    bf16 = ml_dtypes.bfloat16
    g = np.asarray(ln_gamma, np.float64)
    be = np.asarray(ln_beta, np.float64)
    scale = 1.0 / np.sqrt(np.float64(HD))

    def fold(W, b, s=1.0):
        W = np.asarray(W, np.float64)
        b = np.asarray(b, np.float64)
        W_eff = W * g[None, :] * s
        b_eff = (b + W @ be) * s
        wt = np.ascontiguousarray(W_eff.T).astype(bf16)
        return wt, b_eff.astype(np.float32)

    wqt, bq_e = fold(Wq, bq, scale)
    wkt, bk_e = fold(Wk, bk)
    wvt, bv_e = fold(Wv, bv)
    wot = np.ascontiguousarray(np.asarray(Wo, np.float64).T).astype(bf16)
    bo_e = np.asarray(bo, np.float32)

    shared = {
        "wqt": wqt,
        "wkt": wkt,
        "wvt": wvt,
        "wot": wot,
        "bq": np.ascontiguousarray(bq_e.reshape(NDT, P).T),
        "bk": np.ascontiguousarray(bk_e.reshape(NDT, P).T),
        "bv": bv_e.reshape(1, D).astype(np.float32),
        "bo": bo_e.reshape(1, D),
    }
    shared["ident"] = np.eye(P, dtype=bf16)
    x = np.asarray(x, np.float32)
    in_maps = []
    for c in range(NCORES):
        b_idx, half = c // 2, c % 2
        # local query rows first, then the other half of the sequence
        x_local = np.concatenate(
            [
                x[b_idx, half * QROWS : (half + 1) * QROWS],
                x[b_idx, (1 - half) * QROWS : (2 - half) * QROWS],
            ]
        ).astype(bf16)
        in_maps.append({"x": np.ascontiguousarray(x_local), **shared})
    return in_maps


def kernel(x, ln_gamma, ln_beta, Wq, bq, Wk, bk, Wv, bv, Wo, bo):
    nc = _get_program()
    in_maps = _prep_host(x, ln_gamma, ln_beta, Wq, bq, Wk, bk, Wv, bv, Wo, bo)
    res = run_bass_kernel_spmd(nc, in_maps, core_ids=list(range(NCORES)))
    out = np.empty((B, S, D), np.float32)
    for c in range(NCORES):
        b_idx, half = c // 2, c % 2
        out[b_idx, half * QROWS : (half + 1) * QROWS] = res.results[c]["out"]
    return out


if __name__ == "__main__":
    build_program()
    print("program built OK")


# revision 61
# speedup vs baseline: 1.8904x; 1.8904x over previous
"""Fused pre-LN multi-head attention (B=4, S=2048, D=1024, H=16) on 8 trn2 cores.

Sharding: core c -> batch b = c // 2, query-half = c % 2. Each core receives
the FULL 2048-row sequence of its batch (host pre-swapped so the core's own
1024 query rows are always seq tiles 0..7, cast to bf16). The core runs
LayerNorm and K/V projections over all 2048 rows (K/V computed redundantly on
both cores of a batch pair -- cheaper than exchanging K/V via a collective),
the Q projection over its local 1024 rows, attention (16 heads) over the
local queries with full-sequence K/V, and the output projection for its 1024
rows. The host concatenates. No collectives anywhere.

Attention is software-pipelined at seq-tile granularity: while the ctx
matmuls of head-pair t accumulate (two staggered psum groups per head so
probs tiles free at the exp consumption rate), the scores + exp of head-pair
t+1 are emitted, keeping both the PE and Activation engines busy.

LayerNorm gamma/beta and the 1/sqrt(head_dim) scale are folded into the
(host-pre-transposed, bf16) projection weights. Softmax skips max-subtraction
(scores are O(1) by construction); the denominator comes from a ones-column
appended to V inside the probs @ V matmul.
"""

import numpy as np
import ml_dtypes

import concourse.bass as bass
import concourse.mybir as mybir
import concourse.tile as tile
from concourse import bacc
from concourse.bass import ds
from concourse.bass_utils import run_bass_kernel_spmd

F32 = mybir.dt.float32
BF16 = mybir.dt.bfloat16

B, S, D = 4, 2048, 1024
H, HD = 16, 64
EPS = 1e-6
P = 128
NDT = D // P          # 8  d-tiles
NST = S // P          # 16 seq tiles (full batch sequence)
QROWS = S // 2        # 1024 query rows per core
NQT = QROWS // P      # 8
NLT = NST // 2        # 8  local (query) seq tiles
NCORES = 8
HP = H // 2           # 8 head pairs
VSTRIDE = HD + 1      # 65: per-head V columns incl. the ones column


def build_program(nrep=1):
    nc = bacc.Bacc("TRN2", target_bir_lowering=False)

    x_d = nc.dram_tensor("x", [S, D], BF16, kind="ExternalInput")
    wqt_d = nc.dram_tensor("wqt", [D, D], BF16, kind="ExternalInput")
    wkt_d = nc.dram_tensor("wkt", [D, D], BF16, kind="ExternalInput")
    wvt_d = nc.dram_tensor("wvt", [D, D], BF16, kind="ExternalInput")
    wot_d = nc.dram_tensor("wot", [D, D], BF16, kind="ExternalInput")
    id_d = nc.dram_tensor("ident", [P, P], BF16, kind="ExternalInput")
    bq_d = nc.dram_tensor("bq", [P, NDT], F32, kind="ExternalInput")
    bk_d = nc.dram_tensor("bk", [P, NDT], F32, kind="ExternalInput")
    bv_d = nc.dram_tensor("bv", [1, D], F32, kind="ExternalInput")
    bo_d = nc.dram_tensor("bo", [1, D], F32, kind="ExternalInput")
    out_d = nc.dram_tensor("out", [QROWS, D], F32, kind="ExternalOutput")

    sub, mult, add = (
        mybir.AluOpType.subtract,
        mybir.AluOpType.mult,
        mybir.AluOpType.add,
    )
    AF = mybir.ActivationFunctionType

    with tile.TileContext(nc) as tc:
        with (
            tc.tile_pool(name="consts", bufs=1) as consts,
            tc.tile_pool(name="qt", bufs=1) as qt_pool,
            tc.tile_pool(name="kt", bufs=1) as kt_pool,
            tc.tile_pool(name="vp", bufs=1) as v_pool,
            tc.tile_pool(name="ctxt", bufs=1) as ct_pool,
        ):
            eps_t = consts.tile([P, 1], F32)
            nc.vector.memset(eps_t, EPS)
            bq_t = consts.tile([P, NDT], F32)
            bk_t = consts.tile([P, NDT], F32)
            nc.gpsimd.dma_start(out=bq_t, in_=bq_d.ap())
            nc.gpsimd.dma_start(out=bk_t, in_=bk_d.ap())
            bvb = consts.tile([P, D], F32)
            nc.sync.dma_start(out=bvb, in_=bv_d.ap().to_broadcast([P, D]))
            ident = consts.tile([P, P], BF16)
            nc.sync.dma_start(out=ident, in_=id_d.ap())

            # [p, seq_tile, head, 65]; v in cols 0:64, ones column at 64 so
            # the ctx matmul also produces the softmax denominator (row 64).
            V = v_pool.tile([P, NST, H * VSTRIDE], BF16)
            Vr = V.rearrange("p s (h e) -> p s h e", e=VSTRIDE)
            nc.vector.memset(Vr[:, :, :, HD : HD + 1], 1.0)

            QT = qt_pool.tile([P, NDT, QROWS], BF16)
            KT = kt_pool.tile([P, NDT, S], BF16)
            CT = ct_pool.tile([P, NDT, QROWS], BF16)

            # ---- LayerNorm + transpose + V/K/Q projections (full seq) ----
            for _rep in range(nrep):
              with (
                  tc.tile_pool(name="wq", bufs=1) as wq_pool,
                  tc.tile_pool(name="wk", bufs=1) as wk_pool,
                  tc.tile_pool(name="wv", bufs=1) as wv_pool,
                  tc.tile_pool(name="xp", bufs=6) as x_pool,
                  tc.tile_pool(name="xnp", bufs=4) as xn_pool,
                  tc.tile_pool(name="xntp", bufs=1) as xnt_pool,
                  tc.tile_pool(name="statp", bufs=8) as stat_pool,
                  tc.tile_pool(name="psum_proj", bufs=2, space="PSUM") as psum_proj,
                  tc.tile_pool(name="psum_tr", bufs=4, space="PSUM") as psum_tr,
              ):
                  WQ = wq_pool.tile([P, NDT, D], BF16)
                  WK = wk_pool.tile([P, NDT, D], BF16)
                  WV = wv_pool.tile([P, NDT, D], BF16)

                  XNT = xnt_pool.tile([P, NDT, S], BF16)

                  # pre-issue the first x-tile loads so LayerNorm starts
                  # before the weight transfers occupy the DMA engines
                  NPRE = 3
                  xts = []
                  for s in range(NPRE):
                      xt = x_pool.tile([P, D], BF16, name="xt")
                      nc.sync.dma_start(out=xt, in_=x_d.ap()[s * P : (s + 1) * P, :])
                      xts.append(xt)

                  for s in range(NST):
                      if s < NPRE:
                          xt = xts[s]
                      else:
                          xt = x_pool.tile([P, D], BF16, name="xt")
                          nc.sync.dma_start(out=xt, in_=x_d.ap()[s * P : (s + 1) * P, :])
                      st = stat_pool.tile([P, 2, 6], F32)
                      nc.vector.bn_stats(out=st[:, 0], in_=xt[:, 0:512])
                      nc.vector.bn_stats(out=st[:, 1], in_=xt[:, 512:1024])
                      mv = stat_pool.tile([P, 2], F32)
                      nc.vector.bn_aggr(out=mv, in_=st)
                      std = stat_pool.tile([P, 1], F32)
                      nc.scalar.activation(
                          out=std, in_=mv[:, 1:2], func=AF.Sqrt, bias=eps_t
                      )
                      rstd = stat_pool.tile([P, 1], F32)
                      nc.vector.reciprocal(out=rstd, in_=std)
                      xn = xn_pool.tile([P, D], BF16)
                      nc.vector.tensor_scalar(
                          out=xn,
                          in0=xt,
                          scalar1=mv[:, 0:1],
                          scalar2=rstd,
                          op0=sub,
                          op1=mult,
                      )
                      # transpose xn into XNT on the tensor engine (128x128
                      # blocks via identity matmul, 4 blocks batched per psum
                      # tile) with one Act-engine psum->SBUF copy per batch;
                      # keeps the DMA channel free for x and W loads
                      for jh in range(2):
                          trp = psum_tr.tile([P, 4 * P], BF16)
                          for j2 in range(4):
                              j = 4 * jh + j2
                              nc.tensor.transpose(
                                  trp[:, j2 * P : (j2 + 1) * P],
                                  xn[:, j * P : (j + 1) * P],
                                  ident,
                              )
                          nc.scalar.activation(
                              out=XNT[:, 4 * jh : 4 * jh + 4, s * P : (s + 1) * P],
                              in_=trp.rearrange("p (j c) -> p j c", c=P),
                              func=AF.Copy,
                          )
                      if s == 0:
                          for _t in range(NDT):
                              nc.gpsimd.dma_start(
                                  out=WV[:, _t, :],
                                  in_=wvt_d.ap().rearrange("(t p) j -> p t j", p=P)[:, _t, :],
                              )
                      elif s == 1:
                          for _t in range(NDT):
                              nc.gpsimd.dma_start(
                                  out=WK[:, _t, :],
                                  in_=wkt_d.ap().rearrange("(t p) j -> p t j", p=P)[:, _t, :],
                              )
                      elif s == 4:
                          for _t in range(NDT):
                              nc.gpsimd.dma_start(
                                  out=WQ[:, _t, :],
                                  in_=wqt_d.ap().rearrange("(t p) j -> p t j", p=P)[:, _t, :],
                              )
                      # V projection for this seq tile: V[s] = xn[s] @ Wv.T
                      for df in range(2):
                          ps = psum_proj.tile([P, 512], F32)
                          for k in range(NDT):
                              nc.tensor.matmul(
                                  ps,
                                  lhsT=XNT[:, k, s * P : (s + 1) * P],
                                  rhs=WV[:, k, df * 512 : (df + 1) * 512],
                                  start=(k == 0),
                                  stop=(k == NDT - 1),
                              )
                          ps_h = ps.rearrange("p (h e) -> p h e", e=HD)
                          bv_h = bvb[:, df * 512 : (df + 1) * 512].rearrange(
                              "p (h e) -> p h e", e=HD
                          )
                          nc.vector.tensor_tensor(
                              out=Vr[:, s, df * 8 : (df + 1) * 8, 0:HD],
                              in0=ps_h,
                              in1=bv_h,
                              op=add,
                          )
                      if s % 4 == 3:
                          kf = s // 4
                          # K^T chunk (kpos columns kf*512 ..)
                          for i in range(NDT):
                              ps = psum_proj.tile([P, 512], F32)
                              for k in range(NDT):
                                  nc.tensor.matmul(
                                      ps,
                                      lhsT=WK[:, k, i * P : (i + 1) * P],
                                      rhs=XNT[:, k, kf * 512 : (kf + 1) * 512],
                                      start=(k == 0),
                                      stop=(k == NDT - 1),
                                  )
                              nc.vector.tensor_scalar(
                                  out=KT[:, i, kf * 512 : (kf + 1) * 512],
                                  in0=ps,
                                  scalar1=bk_t[:, i : i + 1],
                                  scalar2=None,
                                  op0=add,
                              )
                      if s == NLT - 1:
                          # Q projection (local query rows = tiles 0..7)
                          for i in range(NDT):
                              for qf in range(QROWS // 512):
                                  ps = psum_proj.tile([P, 512], F32)
                                  for k in range(NDT):
                                      nc.tensor.matmul(
                                          ps,
                                          lhsT=WQ[:, k, i * P : (i + 1) * P],
                                          rhs=XNT[:, k, qf * 512 : (qf + 1) * 512],
                                          start=(k == 0),
                                          stop=(k == NDT - 1),
                                      )
                                  nc.vector.tensor_scalar(
                                      out=QT[:, i, qf * 512 : (qf + 1) * 512],
                                      in0=ps,
                                      scalar1=bq_t[:, i : i + 1],
                                      scalar2=None,
                                      op0=add,
                                  )

              # ---- attention (WO prefetched so out-proj starts instantly) --
              with (
                  tc.tile_pool(name="wo", bufs=1) as wo_pool,
                  tc.tile_pool(name="bobp", bufs=1) as bob_pool,
              ):
                WO = wo_pool.tile([P, NDT, D], BF16)
                for _t in range(NDT):
                    nc.sync.dma_start(
                        out=WO[:, _t, :],
                        in_=wot_d.ap().rearrange("(t p) j -> p t j", p=P)[:, _t, :],
                    )
                bob = bob_pool.tile([P, D], F32)
                nc.sync.dma_start(out=bob, in_=bo_d.ap().to_broadcast([P, D]))
                with (
                  tc.tile_pool(name="probs", bufs=32) as probs_pool,
                  tc.tile_pool(name="sep", bufs=2) as se_pool,
                  tc.tile_pool(name="psum_sc", bufs=2, space="PSUM") as psum_sc,
                  tc.tile_pool(name="psum_cx", bufs=4, space="PSUM") as psum_cx,
                ):
                  probs_all = [
                      [[None] * NST for _ in range(2)] for _ in range(HP)
                  ]

                  def emit_scores(t, kt):
                      for hi in range(2):
                          off = hi * HD
                          sps = psum_sc.tile([P, QROWS], F32)
                          for qf in range(QROWS // 512):
                              nc.tensor.matmul(
                                  sps[:, qf * 512 : (qf + 1) * 512],
                                  lhsT=KT[off : off + HD, t, kt * P : (kt + 1) * P],
                                  rhs=QT[off : off + HD, t, qf * 512 : (qf + 1) * 512],
                                  start=True,
                                  stop=True,
                                  tile_position=(off, 0),
                              )
                          pt = probs_pool.tile([P, QROWS], BF16)
                          nc.scalar.activation(out=pt, in_=sps, func=AF.Exp)
                          probs_all[t][hi][kt] = pt

                  def finalize(t, hi, qf, cps):
                      # rows 0..63 = unnormalized ctx, row 64 = sum(exp).
                      # Drain psum immediately (recip + raw-ctx copy) so the
                      # cps slot frees for the next pair without waiting for
                      # the full normalization chain.
                      se = se_pool.tile([P, 512], F32, tag="se")
                      nc.vector.reciprocal(
                          out=se[HD : HD + 1, :],
                          in_=cps[HD : HD + 1, :],
                      )
                      raw = se_pool.tile([HD, 512], BF16, tag="raw", bufs=4)
                      nc.vector.tensor_copy(raw, cps[0:HD, :])
                      # HW partition_broadcast only reads partition 0:
                      # shift the denominator row down first via DMA.
                      se0 = se_pool.tile([1, 512], F32, tag="se0")
                      nc.sync.dma_start(out=se0, in_=se[HD : HD + 1, :])
                      seb = se_pool.tile([P, 512], F32, tag="seb")
                      nc.gpsimd.partition_broadcast(seb[0:HD, :], se0)
                      if hi == 0:
                          nc.vector.tensor_tensor(
                              out=CT[0:HD, t, qf * 512 : (qf + 1) * 512],
                              in0=raw,
                              in1=seb[0:HD, :],
                              op=mult,
                          )
                      else:
                          tmp = se_pool.tile([HD, 512], BF16, tag="ctmp")
                          nc.vector.tensor_tensor(
                              out=tmp,
                              in0=raw,
                              in1=seb[0:HD, :],
                              op=mult,
                          )
                          # partition shift 0..63 -> 64..127 via DMA on the
                          # sync queue (idle during attention)
                          nc.sync.dma_start(
                              out=CT[HD:P, t, qf * 512 : (qf + 1) * 512],
                              in_=tmp,
                          )

                  with tc.tile_pool(name="osb", bufs=2) as osb_pool:

                    def emit_outproj(qt):
                        # shares the psum_sc ring (same tag/shape) -- by the
                        # time out-proj runs, scores emission has stopped, so
                        # the ring slots alternate between out-proj q-tiles
                        ops = psum_sc.tile([P, QROWS], F32, name="sps")
                        for jf in range(2):
                            ps = ops[:, jf * 512 : (jf + 1) * 512]
                            for i in range(NDT):
                                nc.tensor.matmul(
                                    ps,
                                    lhsT=CT[:, i, qt * P : (qt + 1) * P],
                                    rhs=WO[:, i, jf * 512 : (jf + 1) * 512],
                                    start=(i == 0),
                                    stop=(i == NDT - 1),
                                )
                            ot = osb_pool.tile([P, 512], F32)
                            nc.vector.tensor_tensor(
                                out=ot,
                                in0=ps,
                                in1=bob[:, jf * 512 : (jf + 1) * 512],
                                op=add,
                            )
                            nc.sync.dma_start(
                                out=out_d.ap()[
                                    qt * P : (qt + 1) * P,
                                    jf * 512 : (jf + 1) * 512,
                                ],
                                in_=ot,
                            )

                    for kt in range(NST):
                        emit_scores(0, kt)
                    for t in range(HP):
                      probs = probs_all[t]
                      cps = [
                          [
                              psum_cx.tile(
                                  [VSTRIDE, 512], F32,
                                  name=f"cps{_hi}{_qf}", tag="cps",
                              )
                              for _qf in range(2)
                          ]
                          for _hi in range(2)
                      ]
                      # qf=1 trails qf=0 by one step so each probs tile's last
                      # read happens one step after its first, matching the
                      # exp production rate of the next pair's scores.
                      for step in range(NST + 1):
                          if step < NST and t + 1 < HP:
                              emit_scores(t + 1, step)
                          for hi in range(2):
                              for qf, kt in ((0, step), (1, step - 1)):
                                  if 0 <= kt < NST:
                                      nc.tensor.matmul(
                                          cps[hi][qf],
                                          lhsT=Vr[:, kt, 2 * t + hi, :],
                                          rhs=probs[hi][kt][
                                              :, qf * 512 : (qf + 1) * 512
                                          ],
                                          start=(kt == 0),
                                          stop=(kt == NST - 1),
                                      )
                                      if kt == NST - 1:
                                          finalize(t, hi, qf, cps[hi][qf])
                          if t == HP - 1:
                              # overlap out-proj with the last pair's tail:
                              # qf0 columns of CT finalize at step NST-1
                              if step == NST - 1:
                                  for qt in range(NQT // 2):
                                      emit_outproj(qt)
                              elif step == NST:
                                  for qt in range(NQT // 2, NQT):
                                      emit_outproj(qt)

    nc.compile()
    return nc


_NC_CACHE = None


def _get_program():
    global _NC_CACHE
    if _NC_CACHE is None:
        _NC_CACHE = build_program()
    return _NC_CACHE


def _prep_host(x, ln_gamma, ln_beta, Wq, bq, Wk, bk, Wv, bv, Wo, bo):
    bf16 = ml_dtypes.bfloat16
    g = np.asarray(ln_gamma, np.float64)
    be = np.asarray(ln_beta, np.float64)
    scale = 1.0 / np.sqrt(np.float64(HD))

    def fold(W, b, s=1.0):
        W = np.asarray(W, np.float64)
        b = np.asarray(b, np.float64)
        W_eff = W * g[None, :] * s
        b_eff = (b + W @ be) * s
        wt = np.ascontiguousarray(W_eff.T).astype(bf16)
        return wt, b_eff.astype(np.float32)

    wqt, bq_e = fold(Wq, bq, scale)
    wkt, bk_e = fold(Wk, bk)
    wvt, bv_e = fold(Wv, bv)
    wot = np.ascontiguousarray(np.asarray(Wo, np.float64).T).astype(bf16)
    bo_e = np.asarray(bo, np.float32)

    shared = {
        "wqt": wqt,
        "wkt": wkt,
        "wvt": wvt,
        "wot": wot,
        "bq": np.ascontiguousarray(bq_e.reshape(NDT, P).T),
        "bk": np.ascontiguousarray(bk_e.reshape(NDT, P).T),
        "bv": bv_e.reshape(1, D).astype(np.float32),
        "bo": bo_e.reshape(1, D),
    }
    shared["ident"] = np.eye(P, dtype=bf16)
    x = np.asarray(x, np.float32)
    in_maps = []
    for c in range(NCORES):
        b_idx, half = c // 2, c % 2
        # local query rows first, then the other half of the sequence
        x_local = np.concatenate(
            [
                x[b_idx, half * QROWS : (half + 1) * QROWS],
                x[b_idx, (1 - half) * QROWS : (2 - half) * QROWS],
            ]
        ).astype(bf16)
        in_maps.append({"x": np.ascontiguousarray(x_local), **shared})
    return in_maps


def kernel(x, ln_gamma, ln_beta, Wq, bq, Wk, bk, Wv, bv, Wo, bo):
    nc = _get_program()
    in_maps = _prep_host(x, ln_gamma, ln_beta, Wq, bq, Wk, bk, Wv, bv, Wo, bo)
    res = run_bass_kernel_spmd(nc, in_maps, core_ids=list(range(NCORES)))
    out = np.empty((B, S, D), np.float32)
    for c in range(NCORES):
        b_idx, half = c // 2, c % 2
        out[b_idx, half * QROWS : (half + 1) * QROWS] = res.results[c]["out"]
    return out


if __name__ == "__main__":
    build_program()
    print("program built OK")
